# revision 22
# baseline (speedup 1.0000x reference)
"""Trainium2 Bass kernel for the LoRA-mixture layer.

Math (derived from the reference's interleave):  for batch b,
  y[b] = relu( 0.25 * x[b] @ Bcat_b @ Acat_b )
where Bcat_b = concat of adapter_b[4b:4b+4] along rank (rank 16),
      Acat_b = concat of adapter_a[4b:4b+4] along rank.

Sharding: data-parallel, batch b -> core b (8 batches, 8 cores).

The kernel is HBM/fabric-bandwidth bound (x in + y out dominate), so
all device I/O is bf16 (rel-err budget 2e-2 >> bf16's ~4e-3). The host
pre-transposes x[b] to xT [D, S] and packs it so every DMA is 4 KB
contiguous per partition.

Schedule notes (from trace analysis):
 - The SP HWDGE ring wakes ~8.8us into the kernel; the ACT ring wakes
   ~2.5us. The first four (mini) slab loads ride the ACT ring.
 - The out stream caps ~220-300 GB/s, so it must start early and never
   stall: the slab schedule tapers (128-row slabs at both ends) so the
   first y tile is ready ~6us in and the last slab drains quickly.
 - mm2 for slab k is emitted after slab k+1's mm1 so the PE never waits
   on the ACT eviction of hT.
 - Tiny "tickle" matmuls on const data keep the PE activity monitor
   from re-throttling the clock (1.2 vs 2.4 GHz) during DMA stalls.

Per-slab dataflow (W rows; W=512 slabs load in 4 quarter sub-DMAs):
  DMA in xt sub-slab  [128p, c, W s] bf16 (0.5 MB)
  mm1: hT4[128, W] += bcat4[128,128].T @ xtChunk[128,W]  per d-chunk
       (bcat4 has Bcat replicated at column offsets 0/32/64/96 so hT
        lands replicated at partition offsets 0/32/64/96)
  ACT-evict hT4 -> SBUF bf16
  mm2: per s-subtile t, 4 concurrent row-group matmuls (tile_position):
       y[128,512] = hT[16,128].T @ Acat[16,512]  (0.25 folded into Acat)
  relu-evict PSUM -> SBUF bf16 (split DVE / ACT)
  DMA out y tile [128p, 2048d] bf16 (0.5 MB) per s-subtile
"""

import numpy as np
import ml_dtypes

import concourse.bass as bass
import concourse.mybir as mybir
import concourse.tile as tile
from concourse import bacc
from concourse.bass_utils import run_bass_kernel_spmd

B, S, D = 8, 4096, 2048
R = 16               # concatenated rank per batch (4 adapters x rank 4)
N_CORES = 8
DC = D // 128        # 16 contraction chunks
NDP = D // 512       # 4 output-column chunks

# slab plan: (rows, n_sub_dmas)
SLABS = [(512, 4)] * 8
assert sum(w for w, _ in SLABS) == S
NSLAB = len(SLABS)
N_ACT_RING_DMAS = 0  # first N sub-DMAs ride the early-waking ACT ring

BF16 = mybir.dt.bfloat16
F32 = mybir.dt.float32
NPBF16 = ml_dtypes.bfloat16
RELU = mybir.ActivationFunctionType.Relu


def build_nc():
    nc = bacc.Bacc("TRN2", target_bir_lowering=False, debug=False)

    # xt: x[b].T packed per sub-DMA as [p, c, s'] blocks, flat buffer
    xt = nc.dram_tensor("xt", [S * D], BF16, kind="ExternalInput")
    # bcat4 packed p-major on host: [p, c, r] with d = c*128 + p. Bcat
    # columns replicated at offsets 0/32/64/96 (zeros elsewhere) so mm1
    # emits hT at 4 partition offsets for row-packed mm2.
    bcat4 = nc.dram_tensor("bcat4", [128, DC, 128], BF16, kind="ExternalInput")
    # acat4 [128, D]: acat rows replicated at partition offsets 0/32/64/96
    acat4 = nc.dram_tensor("acat4", [128, D], BF16, kind="ExternalInput")
    # y [s, d] flat; tiles written per 128 rows
    y = nc.dram_tensor("y", [S * D], BF16, kind="ExternalOutput")

    with tile.TileContext(nc) as tc:
        with (
            tc.tile_pool(name="const", bufs=1) as cpool,
            tc.tile_pool(name="xin", bufs=6) as xin_pool,
            tc.tile_pool(name="ht", bufs=3) as ht_pool,
            tc.tile_pool(name="yout", bufs=6) as y_pool,
            tc.tile_pool(name="ph", bufs=2, space="PSUM") as ph_pool,
            tc.tile_pool(name="py", bufs=2, space="PSUM") as py_pool,
            tc.tile_pool(name="ptk", bufs=1, space="PSUM") as ptk_pool,
        ):
            bcat_sb = cpool.tile([128, DC, 128], BF16)
            nc.scalar.dma_start(out=bcat_sb[:], in_=bcat4.ap())
            acat_rep = cpool.tile([128, D], BF16)
            nc.scalar.dma_start(out=acat_rep[:], in_=acat4.ap())

            # HAM tickle target: a tiny matmul on const data keeps the PE
            # activity monitor from re-throttling the clock during DMA
            # stalls. The scratch PSUM tile is never read.
            ptick = ptk_pool.tile([128, 64], F32)

            def tickle():
                nc.tensor.matmul(
                    ptick[:], bcat_sb[:, 0, :], bcat_sb[:, 0, :64],
                    start=True, stop=True,
                )

            ht_reps = [None] * NSLAB
            slab_s0 = []
            s0 = 0
            for w, _ in SLABS:
                slab_s0.append(s0)
                s0 += w

            def emit_mm2(k):
                # mm2 for slab k: per s-subtile t, 4 concurrent row-group
                # matmuls (row group j = d'-chunk), relu-evict, DMA out.
                w, _ = SLABS[k]
                for t in range(w // 128):
                    tickle()
                    y_sb = y_pool.tile([128, D], BF16, tag="yout")
                    # mm2 lands in two 2-bank PSUM tiles so the relu evict
                    # runs as two wide [128,1024] ops (one DVE, one ACT)
                    # instead of four -- fewer ops and sem hops per y tile.
                    pys = []
                    for jh in range(2):
                        py = py_pool.tile([128, 1024], F32, tag="py")
                        for j2 in range(2):
                            j = 2 * jh + j2
                            nc.tensor.matmul(
                                py[:, j2 * 512 : (j2 + 1) * 512],
                                ht_reps[k][
                                    32 * j : 32 * j + R, t * 128 : (t + 1) * 128
                                ],
                                acat_rep[32 * j : 32 * j + R, j * 512 : (j + 1) * 512],
                                start=True,
                                stop=True,
                                tile_position=(32 * j, 0),
                            )
                        pys.append(py)
                    nc.vector.tensor_scalar_max(y_sb[:, 0:1024], pys[0][:], 0.0)
                    nc.scalar.activation(y_sb[:, 1024:2048], pys[1][:], RELU)
                    off = (slab_s0[k] + t * 128) * D
                    # Two out queues (gpsimd + ACT HWDGE) vs one in queue:
                    # the per-SDMA-engine round-robin then gives the out
                    # stream ~2/3 of the wire. The merged evicts above keep
                    # the ACT engine light enough to absorb the DMA issues.
                    eng = nc.gpsimd if t % 2 == 0 else nc.scalar
                    eng.dma_start(
                        out=y.ap()[off : off + 128 * D].rearrange(
                            "(p d) -> p d", p=128
                        ),
                        in_=y_sb[:],
                    )

            xoff = 0
            ndma = 0
            for sl, (w, nsub) in enumerate(SLABS):
                csz = DC // nsub
                ht_ps = ph_pool.tile([128, 512], F32, tag="ph")
                for i in range(nsub):
                    x_sb = xin_pool.tile([128, csz, w], BF16, tag="xin")
                    eng = nc.scalar if ndma < N_ACT_RING_DMAS else nc.sync
                    eng.dma_start(
                        out=x_sb[:],
                        in_=xt.ap()[xoff : xoff + 128 * csz * w].rearrange(
                            "(p c s) -> p c s", p=128, c=csz
                        ),
                    )
                    xoff += 128 * csz * w
                    ndma += 1
                    tickle()
                    for cc in range(csz):
                        nc.tensor.matmul(
                            ht_ps[:, :w],
                            bcat_sb[:, i * csz + cc, :],
                            x_sb[:, cc, :],
                            start=(i == 0 and cc == 0),
                            stop=(i == nsub - 1 and cc == csz - 1),
                        )
                ht_rep = ht_pool.tile([128, 512], BF16, tag="ht")
                nc.scalar.copy(ht_rep[:, :w], ht_ps[:, :w])
                ht_reps[sl] = ht_rep
                # mm2 lags one slab behind mm1 so the PE never waits on the
                # ACT eviction of hT.
                if sl >= 1:
                    emit_mm2(sl - 1)
            emit_mm2(NSLAB - 1)

    nc.compile()
    return nc


_NC = None


def _get_nc():
    global _NC
    if _NC is None:
        _NC = build_nc()
    return _NC


def pack_xt(xb):
    """x[b] [S, D] f32 -> flat bf16 buffer matching the kernel's slab plan."""
    xT = xb.T  # [D, S]
    blocks = []
    s0 = 0
    for w, nsub in SLABS:
        csz = DC // nsub
        # [D, w] -> [nsub, csz, 128, w] -> [nsub, 128(p), csz, w]
        blk = xT[:, s0 : s0 + w].reshape(nsub, csz, 128, w).transpose(0, 2, 1, 3)
        blocks.append(np.ascontiguousarray(blk).reshape(-1))
        s0 += w
    return np.concatenate(blocks).astype(NPBF16)


def make_in_maps(x, adapter_b, adapter_a):
    in_maps = []
    for b in range(B):
        bc = np.ascontiguousarray(
            adapter_b[4 * b : 4 * b + 4].transpose(1, 0, 2).reshape(D, R)
        ).astype(np.float32)
        bc4 = np.zeros((D, 128), dtype=np.float32)
        for j in range(4):
            bc4[:, 32 * j : 32 * j + R] = bc
        # pack p-major: [D, 128] -> [p, c, r] with d = c*128 + p
        bc4 = np.ascontiguousarray(bc4.reshape(DC, 128, 128).transpose(1, 0, 2))
        ac = np.ascontiguousarray(
            adapter_a[4 * b : 4 * b + 4].reshape(R, D) * 0.25
        ).astype(np.float32)
        ac4 = np.zeros((128, D), dtype=np.float32)
        for j in range(4):
            ac4[32 * j : 32 * j + R, :] = ac
        in_maps.append(
            {
                "xt": pack_xt(x[b]),
                "bcat4": bc4.astype(NPBF16),
                "acat4": ac4.astype(NPBF16),
            }
        )
    return in_maps


def run(x, adapter_b, adapter_a, **run_kwargs):
    nc = _get_nc()
    in_maps = make_in_maps(x, adapter_b, adapter_a)
    res = run_bass_kernel_spmd(nc, in_maps, list(range(N_CORES)), **run_kwargs)
    out = np.stack(
        [
            res.results[i]["y"].reshape(S, D).astype(np.float32)
            for i in range(N_CORES)
        ]
    )
    return out, res


def kernel(x, adapter_b, adapter_a):
    out, _ = run(x, adapter_b, adapter_a)
    return out


# revision 24
# speedup vs baseline: 1.2016x; 1.2016x over previous
"""Trainium2 Bass kernel for the LoRA-mixture layer.

Math (derived from the reference's interleave):  for batch b,
  y[b] = relu( 0.25 * x[b] @ Bcat_b @ Acat_b )
where Bcat_b = concat of adapter_b[4b:4b+4] along rank (rank 16),
      Acat_b = concat of adapter_a[4b:4b+4] along rank.

Sharding: data-parallel, batch b -> core b (8 batches, 8 cores).

The kernel is HBM/fabric-bandwidth bound (x in + y out dominate), so
all device I/O is bf16 (rel-err budget 2e-2 >> bf16's ~4e-3). The host
pre-transposes x[b] to xT [D, S] and packs it so every DMA is 4 KB
contiguous per partition.

Schedule notes (from trace analysis):
 - The SP HWDGE ring wakes ~8.8us into the kernel; the ACT ring wakes
   ~2.5us. The first four (mini) slab loads ride the ACT ring.
 - The out stream caps ~220-300 GB/s, so it must start early and never
   stall: the slab schedule tapers (128-row slabs at both ends) so the
   first y tile is ready ~6us in and the last slab drains quickly.
 - mm2 for slab k is emitted after slab k+1's mm1 so the PE never waits
   on the ACT eviction of hT.
 - Tiny "tickle" matmuls on const data keep the PE activity monitor
   from re-throttling the clock (1.2 vs 2.4 GHz) during DMA stalls.

Per-slab dataflow (W rows; W=512 slabs load in 4 quarter sub-DMAs):
  DMA in xt sub-slab  [128p, c, W s] bf16 (0.5 MB)
  mm1: hT4[128, W] += bcat4[128,128].T @ xtChunk[128,W]  per d-chunk
       (bcat4 has Bcat replicated at column offsets 0/32/64/96 so hT
        lands replicated at partition offsets 0/32/64/96)
  ACT-evict hT4 -> SBUF bf16
  mm2: per s-subtile t, 4 concurrent row-group matmuls (tile_position):
       y[128,512] = hT[16,128].T @ Acat[16,512]  (0.25 folded into Acat)
  relu-evict PSUM -> SBUF bf16 (split DVE / ACT)
  DMA out y tile [128p, 2048d] bf16 (0.5 MB) per s-subtile
"""

import numpy as np
import ml_dtypes

import concourse.bass as bass
import concourse.mybir as mybir
import concourse.tile as tile
from concourse import bacc
from concourse.bass_utils import run_bass_kernel_spmd

B, S, D = 8, 4096, 2048
R = 16               # concatenated rank per batch (4 adapters x rank 4)
N_CORES = 8
DC = D // 128        # 16 contraction chunks
NDP = D // 512       # 4 output-column chunks

# slab plan: (rows, n_sub_dmas)
SLABS = [(512, 4)] * 8
assert sum(w for w, _ in SLABS) == S
NSLAB = len(SLABS)
N_ACT_RING_DMAS = 0  # first N sub-DMAs ride the early-waking ACT ring

BF16 = mybir.dt.bfloat16
F32 = mybir.dt.float32
NPBF16 = ml_dtypes.bfloat16
RELU = mybir.ActivationFunctionType.Relu


def build_nc():
    nc = bacc.Bacc("TRN2", target_bir_lowering=False, debug=False)

    # xt: x[b].T packed per sub-DMA as [p, c, s'] blocks, flat buffer
    xt = nc.dram_tensor("xt", [S * D], BF16, kind="ExternalInput")
    # bcat4 packed p-major on host: [p, c, r] with d = c*128 + p. Bcat
    # columns replicated at offsets 0/32/64/96 (zeros elsewhere) so mm1
    # emits hT at 4 partition offsets for row-packed mm2.
    bcat4 = nc.dram_tensor("bcat4", [128, DC, 128], BF16, kind="ExternalInput")
    # acat4 [128, D]: acat rows replicated at partition offsets 0/32/64/96
    acat4 = nc.dram_tensor("acat4", [128, D], BF16, kind="ExternalInput")
    # y [s, d] flat; tiles written per 128 rows
    y = nc.dram_tensor("y", [S * D], BF16, kind="ExternalOutput")

    with tile.TileContext(nc) as tc:
        with (
            tc.tile_pool(name="const", bufs=1) as cpool,
            tc.tile_pool(name="xin", bufs=6) as xin_pool,
            tc.tile_pool(name="ht", bufs=3) as ht_pool,
            tc.tile_pool(name="yout", bufs=6) as y_pool,
            tc.tile_pool(name="ph", bufs=2, space="PSUM") as ph_pool,
            tc.tile_pool(name="py", bufs=4, space="PSUM") as py_pool,
            tc.tile_pool(name="ptk", bufs=1, space="PSUM") as ptk_pool,
        ):
            bcat_sb = cpool.tile([128, DC, 128], BF16)
            nc.scalar.dma_start(out=bcat_sb[:], in_=bcat4.ap())
            acat_rep = cpool.tile([128, D], BF16)
            nc.scalar.dma_start(out=acat_rep[:], in_=acat4.ap())

            # HAM tickle target: a tiny matmul on const data keeps the PE
            # activity monitor from re-throttling the clock during DMA
            # stalls. The scratch PSUM tile is never read.
            ptick = ptk_pool.tile([128, 64], F32)

            def tickle():
                nc.tensor.matmul(
                    ptick[:], bcat_sb[:, 0, :], bcat_sb[:, 0, :64],
                    start=True, stop=True,
                )

            ht_reps = [None] * NSLAB
            slab_s0 = []
            s0 = 0
            for w, _ in SLABS:
                slab_s0.append(s0)
                s0 += w

            def emit_mm2(k):
                # mm2 for slab k: per s-subtile t, 4 concurrent row-group
                # matmuls (row group j = d'-chunk), relu-evict, DMA out.
                w, _ = SLABS[k]
                for t in range(w // 128):
                    tickle()
                    y_sb = y_pool.tile([128, D], BF16, tag="yout")
                    pys = []
                    for j in range(NDP):
                        py = py_pool.tile([128, 512], F32, tag="py")
                        nc.tensor.matmul(
                            py[:],
                            ht_reps[k][32 * j : 32 * j + R, t * 128 : (t + 1) * 128],
                            acat_rep[32 * j : 32 * j + R, j * 512 : (j + 1) * 512],
                            start=True,
                            stop=True,
                            tile_position=(32 * j, 0),
                        )
                        pys.append(py)
                    for j in range(NDP):
                        dst = y_sb[:, j * 512 : (j + 1) * 512]
                        if j < 2:
                            nc.vector.tensor_scalar_max(dst, pys[j][:], 0.0)
                        else:
                            nc.scalar.activation(dst, pys[j][:], RELU)
                    off = (slab_s0[k] + t * 128) * D
                    nc.gpsimd.dma_start(
                        out=y.ap()[off : off + 128 * D].rearrange(
                            "(p d) -> p d", p=128
                        ),
                        in_=y_sb[:],
                    )

            xoff = 0
            ndma = 0
            for sl, (w, nsub) in enumerate(SLABS):
                csz = DC // nsub
                ht_ps = ph_pool.tile([128, 512], F32, tag="ph")
                for i in range(nsub):
                    x_sb = xin_pool.tile([128, csz, w], BF16, tag="xin")
                    eng = nc.scalar if ndma < N_ACT_RING_DMAS else nc.sync
                    eng.dma_start(
                        out=x_sb[:],
                        in_=xt.ap()[xoff : xoff + 128 * csz * w].rearrange(
                            "(p c s) -> p c s", p=128, c=csz
                        ),
                    )
                    xoff += 128 * csz * w
                    ndma += 1
                    tickle()
                    for cc in range(csz):
                        nc.tensor.matmul(
                            ht_ps[:, :w],
                            bcat_sb[:, i * csz + cc, :],
                            x_sb[:, cc, :],
                            start=(i == 0 and cc == 0),
                            stop=(i == nsub - 1 and cc == csz - 1),
                        )
                ht_rep = ht_pool.tile([128, 512], BF16, tag="ht")
                nc.scalar.copy(ht_rep[:, :w], ht_ps[:, :w])
                ht_reps[sl] = ht_rep
                # mm2 lags one slab behind mm1 so the PE never waits on the
                # ACT eviction of hT.
                if sl >= 1:
                    emit_mm2(sl - 1)
            emit_mm2(NSLAB - 1)

    nc.compile()
    return nc


_NC = None


def _get_nc():
    global _NC
    if _NC is None:
        _NC = build_nc()
    return _NC


def pack_xt(xb):
    """x[b] [S, D] f32 -> flat bf16 buffer matching the kernel's slab plan."""
    xT = xb.T  # [D, S]
    blocks = []
    s0 = 0
    for w, nsub in SLABS:
        csz = DC // nsub
        # [D, w] -> [nsub, csz, 128, w] -> [nsub, 128(p), csz, w]
        blk = xT[:, s0 : s0 + w].reshape(nsub, csz, 128, w).transpose(0, 2, 1, 3)
        blocks.append(np.ascontiguousarray(blk).reshape(-1))
        s0 += w
    return np.concatenate(blocks).astype(NPBF16)


def make_in_maps(x, adapter_b, adapter_a):
    in_maps = []
    for b in range(B):
        bc = np.ascontiguousarray(
            adapter_b[4 * b : 4 * b + 4].transpose(1, 0, 2).reshape(D, R)
        ).astype(np.float32)
        bc4 = np.zeros((D, 128), dtype=np.float32)
        for j in range(4):
            bc4[:, 32 * j : 32 * j + R] = bc
        # pack p-major: [D, 128] -> [p, c, r] with d = c*128 + p
        bc4 = np.ascontiguousarray(bc4.reshape(DC, 128, 128).transpose(1, 0, 2))
        ac = np.ascontiguousarray(
            adapter_a[4 * b : 4 * b + 4].reshape(R, D) * 0.25
        ).astype(np.float32)
        ac4 = np.zeros((128, D), dtype=np.float32)
        for j in range(4):
            ac4[32 * j : 32 * j + R, :] = ac
        in_maps.append(
            {
                "xt": pack_xt(x[b]),
                "bcat4": bc4.astype(NPBF16),
                "acat4": ac4.astype(NPBF16),
            }
        )
    return in_maps


def run(x, adapter_b, adapter_a, **run_kwargs):
    nc = _get_nc()
    in_maps = make_in_maps(x, adapter_b, adapter_a)
    res = run_bass_kernel_spmd(nc, in_maps, list(range(N_CORES)), **run_kwargs)
    out = np.stack(
        [
            res.results[i]["y"].reshape(S, D).astype(np.float32)
            for i in range(N_CORES)
        ]
    )
    return out, res


def kernel(x, adapter_b, adapter_a):
    out, _ = run(x, adapter_b, adapter_a)
    return out
